# revision 39
# baseline (speedup 1.0000x reference)
"""nn_DMSAttentionWrapper kernel for Trainium2, 8 NeuronCores.

The reference's eviction/causal mask is `jnp.maximum(dms, causal)` where the
two -inf regions are disjoint (dms: q > k+WIN and evicted; causal: q < k), so
the combined additive mask is identically zero: the oracle computes *dense,
non-causal, unmasked* multi-head attention.  The decision head (Wd, bd) does
not affect the output at all.

Sharding: data-parallel over batch (2) x tensor-parallel over heads (4 groups
of 4 heads).  Per-core schedule (latency-shaped around two facts: the ScalarE
exp wall of ~18us per (q-chunk, head) group vs ~14.6us of S+PV matmul work,
and a per-core input stream of only ~210GB/s because core pairs share a
device's HBM):

  Phase A: 8 warmup matmuls ramp the PE p-state while the first DMAs land
  (xt via the GpSimd queue in parallel with weights on Sync; xt0 split in 4
  chunks).  Q/K projection chains 0-9 are TWO-PASS (dt 0-7 into a bf16
  partial, dt 8-15 merged on DVE) so resident re-reads fill the xt arrival
  window instead of stalling the 4-wide dt-major ramp; K chains for
  (h2, qh1) and all of h3 are deferred into phase B.  V projection reuses
  the same psum pool/tag (no pool-transition drain).

  Phase B: per group: S^T into 2-bank PSUM, one exp ACTIVATE over
  [128,1024], exp tiles accumulated into the softmax denominator on DVE
  (GpSimd tensor ops are ~3x slower and cannot read PSUM), ones-matmul
  denominator AFTER the PV tail (no head-of-line block on the esum chain),
  reciprocal_approx_fast, normalize on DVE.  The exp-wall PE idle of groups
  0-3 is filled with the deferred K units (with emission-order deadline
  drains); groups 4-7 interleave block-0 Wo chains every other step (casts
  mostly DVE, every 3rd on ScalarE which has slack under the exp wall); the
  block-1 Wo tail runs with 4 psum bufs after releasing the group pools
  (bridged by 6 chains to hide the transition), casts alternating
  DVE/ScalarE, and the last two chains cast+DMA in halves to cut the end
  drain.  Host sums 4 bf16 partials per batch.
SCALE is folded into Wq on the host.
"""

import numpy as np
from collections import deque
from contextlib import ExitStack

import ml_dtypes
import concourse.bass as bass
import concourse.tile as tile
from concourse import bacc, mybir
from concourse.bass_utils import run_bass_kernel_spmd

B, T, D, H = 2, 2048, 2048, 16
HD = 128
NCORES = 8
CPB = NCORES // B          # cores per batch
HPC = H // CPB             # heads per core
HS = HPC * HD              # head-slice width (columns of Wq/Wk/Wv, rows of Wo)
SCALE = 1.0 / float(np.sqrt(HD))

F32 = mybir.dt.float32
BF16 = mybir.dt.bfloat16

P = 128                    # partition dim
NF = 512                   # matmul free dim / psum bank (fp32)
QH = 1024                  # q macro-chunk (exp ACTIVATE free dim, 2 psum banks)
NQH = T // QH              # 2 q macro-chunks
DT = D // P                # 16 contraction tiles over D
KT = T // P                # 16 k tiles
TT = T // P                # 16 t tiles
LEAD = 2                   # S/exp lead over PV/denom in the kt pipeline
NWARM = 8                  # warmup matmuls to ramp PE p-state during DMA wait

ALU = mybir.AluOpType
ACTF = mybir.ActivationFunctionType

_CACHE: dict = {}


def _build():
    if "nc" in _CACHE:
        return _CACHE["nc"]

    nc = bacc.Bacc("TRN2", target_bir_lowering=False, debug=False)

    xT = nc.dram_tensor("xT", [D, T], BF16, kind="ExternalInput").ap()
    wq_d = nc.dram_tensor("wq", [D, HS], BF16, kind="ExternalInput").ap()
    wk_d = nc.dram_tensor("wk", [D, HS], BF16, kind="ExternalInput").ap()
    wv_d = nc.dram_tensor("wv", [D, HS], BF16, kind="ExternalInput").ap()
    wo_d = nc.dram_tensor("wo", [HS, D], BF16, kind="ExternalInput").ap()
    out_d = nc.dram_tensor("out", [T, D], BF16, kind="ExternalOutput").ap()

    with tile.TileContext(nc) as tc, ExitStack() as ctx:
        const_pool = ctx.enter_context(tc.tile_pool(name="const", bufs=1))
        qk_pool = ctx.enter_context(tc.tile_pool(name="qk", bufs=1))
        v_pool = ctx.enter_context(tc.tile_pool(name="v", bufs=1))
        wo_pool = ctx.enter_context(tc.tile_pool(name="wo", bufs=1))
        xt_pool = ctx.enter_context(tc.tile_pool(name="xt", bufs=1))
        wk23_pool = ctx.enter_context(tc.tile_pool(name="wk23", bufs=1))

        # warm memset FIRST: the warmup matmuls depend only on it
        warm = const_pool.tile([P, NF], BF16, name="warm")
        nc.vector.memset(warm[:], 0.03125)
        ones_f32 = const_pool.tile([P, P], F32, name="ones_f32")
        nc.vector.memset(ones_f32[:], 1.0)
        ones128 = const_pool.tile([P, P], BF16, name="ones128")
        nc.vector.tensor_copy(ones128[:], ones_f32[:])

        # resident Q/K (head-dim x T, bf16) and V (T x 4*HD, bf16)
        qall = [qk_pool.tile([P, T], BF16, name=f"qh{h}") for h in range(HPC)]
        kall = [qk_pool.tile([P, T], BF16, name=f"kh{h}") for h in range(HPC)]
        vall = [v_pool.tile([P, HS], BF16, name=f"vt{i}") for i in range(KT)]
        # x^T tiles stay resident through phase B (deferred K chains read them)
        xt = [xt_pool.tile([P, T], BF16, name=f"xt{i}") for i in range(DT)]
        # wk columns for heads 2..3 also persist into phase B
        wk23 = [wk23_pool.tile([P, 2 * HD], BF16, name=f"wk23_{i}")
                for i in range(DT)]

        # =================== Phase A: projections ===================
        with ExitStack() as actx:
            w_pool = actx.enter_context(tc.tile_pool(name="w", bufs=1))
            ps_a = actx.enter_context(tc.tile_pool(name="ps_a", bufs=4, space="PSUM"))

            # warmup: ramp the PE p-state while the first input DMAs land
            # (depends only on the warm memset, not on ones128)
            for _ in range(NWARM):
                pswm = ps_a.tile([P, QH], F32, name="psqk", tag="a")
                nc.tensor.matmul(pswm[:, 0:NF], warm[:, 0:P], warm[:],
                                 start=True, stop=True)

            # xt DMAs on the GpSimd queue, weights on Sync: the two streams
            # transfer in parallel.  xt0 split in 4 chunks so the first ramp
            # chain's first matmul only waits for wq0 + 256KB of xt.
            for c in range(4):
                nc.gpsimd.dma_start(xt[0][:, c * NF:(c + 1) * NF],
                                    xT[0:P, c * NF:(c + 1) * NF])
            for i in range(1, DT):
                nc.gpsimd.dma_start(xt[i][:], xT[i * P:(i + 1) * P, :])

            wq_t, wk01_t, wv_t = [], [], []
            for i in range(DT):
                t = w_pool.tile([P, HS], BF16, name=f"wq{i}")
                nc.sync.dma_start(t[:], wq_d[i * P:(i + 1) * P, :])
                wq_t.append(t)
            for i in range(DT):
                t = w_pool.tile([P, 2 * HD], BF16, name=f"wk01_{i}")
                nc.sync.dma_start(t[:], wk_d[i * P:(i + 1) * P, 0:2 * HD])
                wk01_t.append(t)
            for i in range(DT):
                nc.sync.dma_start(wk23[i][:], wk_d[i * P:(i + 1) * P, 2 * HD:])
            for i in range(DT):
                t = w_pool.tile([P, HS], BF16, name=f"wv{i}")
                nc.sync.dma_start(t[:], wv_d[i * P:(i + 1) * P, :])
                wv_t.append(t)
            wo_sb = []
            for h in range(HPC):
                t = wo_pool.tile([P, D], BF16, name=f"wo{h}")
                nc.sync.dma_start(t[:], wo_d[h * HD:(h + 1) * HD, :])
                wo_sb.append(t)

            def wk_ap(dt_i, h):
                if h < 2:
                    return wk01_t[dt_i][:, h * HD:(h + 1) * HD]
                return wk23[dt_i][:, (h - 2) * HD:(h - 1) * HD]

            # Q/K projections: per (h, q-macro-chunk), one [128,1024] psum.
            # Only the K chain for (h3, qh1) is deferred into phase B.
            # The per-core input stream lands at only ~210GB/s (two cores
            # share a device's HBM), so xt tiles arrive ~2.4us apart while
            # 8 psum banks cap first-pass consumption at 1.71us/dt.  To
            # avoid pacing stalls, chains 0..SPLIT-1 are TWO-PASS: dt 0-7
            # into a bf16 partial, then dt 8-15 merged on DVE -- the
            # resident re-reads of part 1 fill the dt 8-15 arrival window.
            chains = [("q", h, qh) for h in range(HPC) for qh in range(NQH)]
            chains += [("k", 0, 0), ("k", 0, 1), ("k", 1, 0), ("k", 1, 1),
                       ("k", 2, 0)]
            SPLIT = 10
            HDT = DT // 2

            part_pool = actx.enter_context(tc.tile_pool(name="part", bufs=1))
            part1 = {}

            def qk_mm(ps, kind, h, qh, dt_i, start, stop):
                wt = wq_t[dt_i][:, h * HD:(h + 1) * HD] if kind == "q" \
                    else wk_ap(dt_i, h)
                for hf in range(2):
                    nc.tensor.matmul(
                        ps[:, hf * NF:(hf + 1) * NF],
                        wt,
                        xt[dt_i][:, qh * QH + hf * NF:qh * QH + (hf + 1) * NF],
                        start=start, stop=stop,
                    )

            def qk_dst(ci):
                kind, h, qh = chains[ci]
                dst = qall[h] if kind == "q" else kall[h]
                return dst[:, qh * QH:(qh + 1) * QH]

            # part-1 ramp: chains 0-3 dt-major over dt 0-7
            NRAMP = 4
            ramp_ps = [ps_a.tile([P, QH], F32, name="psqk", tag="a")
                       for _ in range(NRAMP)]
            for dt_i in range(HDT):
                for ci in range(NRAMP):
                    kind, h, qh = chains[ci]
                    qk_mm(ramp_ps[ci], kind, h, qh, dt_i,
                          start=(dt_i == 0), stop=(dt_i == HDT - 1))
            for ci in range(NRAMP):
                pt = part_pool.tile([P, QH], BF16, name=f"p1_{ci}")
                nc.vector.tensor_copy(pt[:], ramp_ps[ci][:])
                part1[ci] = pt

            # part-1 of chains 4..SPLIT-1 (dt 0-7 resident)
            for ci in range(NRAMP, SPLIT):
                kind, h, qh = chains[ci]
                ps = ps_a.tile([P, QH], F32, name="psqk", tag="a")
                for dt_i in range(HDT):
                    qk_mm(ps, kind, h, qh, dt_i,
                          start=(dt_i == 0), stop=(dt_i == HDT - 1))
                pt = part_pool.tile([P, QH], BF16, name=f"p1_{ci}")
                nc.vector.tensor_copy(pt[:], ps[:])
                part1[ci] = pt

            # full chains SPLIT..end (all xt resident by now)
            for ci in range(SPLIT, len(chains)):
                kind, h, qh = chains[ci]
                ps = ps_a.tile([P, QH], F32, name="psqk", tag="a")
                for dt_i in range(DT):
                    qk_mm(ps, kind, h, qh, dt_i,
                          start=(dt_i == 0), stop=(dt_i == DT - 1))
                nc.vector.tensor_copy(qk_dst(ci), ps[:])

            # part-2 of chains 0..SPLIT-1 (dt 8-15) + merge with part-1
            # (cast to bf16 first: a mixed f32-psum + bf16 tensor_tensor
            # takes a ~8.5us slow path on DVE)
            for ci in range(SPLIT):
                kind, h, qh = chains[ci]
                ps = ps_a.tile([P, QH], F32, name="psqk", tag="a")
                for dt_i in range(HDT, DT):
                    qk_mm(ps, kind, h, qh, dt_i,
                          start=(dt_i == HDT), stop=(dt_i == DT - 1))
                p2 = part_pool.tile([P, QH], BF16, name="p2", tag="p2", bufs=2)
                nc.vector.tensor_copy(p2[:], ps[:])
                nc.vector.tensor_add(qk_dst(ci), p2[:], part1[ci][:])

            # V projection reuses the same psum pool/tag (no pool-transition
            # drain; 4-deep rotation gives ample copy overlap)
            for tt_i in range(TT):
                ps = ps_a.tile([P, NF], F32, name="psv", tag="a")
                for dt_i in range(DT):
                    nc.tensor.matmul(
                        ps[:], xt[dt_i][:, tt_i * P:(tt_i + 1) * P],
                        wv_t[dt_i][:],
                        start=(dt_i == 0), stop=(dt_i == DT - 1),
                    )
                nc.vector.tensor_copy(vall[tt_i][:], ps[:])

        # =================== Phase B: attention + Wo ===================
        with ExitStack() as bctx:
            e_pool = bctx.enter_context(tc.tile_pool(name="e", bufs=5))
            es_pool = bctx.enter_context(tc.tile_pool(name="es", bufs=4))
            r_pool = bctx.enter_context(tc.tile_pool(name="r", bufs=2))
            ot_pool = bctx.enter_context(tc.tile_pool(name="ot", bufs=1))
            obounce = bctx.enter_context(tc.tile_pool(name="obounce", bufs=4))
            ps_w = bctx.enter_context(tc.tile_pool(name="ps_w", bufs=2, space="PSUM"))

            # one ot tile per (head, q-block) so Wo of block 0 never picks up
            # a false dependency on block-1 normalize writes
            ot = {(h, qh): ot_pool.tile([P, QH], BF16, name=f"ot{h}_{qh}")
                  for h in range(HPC) for qh in range(NQH)}

            # ---- deferred K projection units (fill exp-bound groups 0-3) ----
            # each (h, cr) is a 16-matmul chain into a [128,512] psum plus a
            # final copy into kall[h][:, cr*512 : (cr+1)*512]
            k_q = deque()

            def make_k_units(h, cr):
                st = {}
                for dt_i in range(DT):
                    def u(dt_i=dt_i, h=h, cr=cr, st=st):
                        if dt_i == 0:
                            st["ps"] = ps_w.tile([P, NF], F32, name="pw", tag="w")
                        nc.tensor.matmul(
                            st["ps"][:], wk23[dt_i][:, (h - 2) * HD:(h - 1) * HD],
                            xt[dt_i][:, cr * NF:(cr + 1) * NF],
                            start=(dt_i == 0), stop=(dt_i == DT - 1),
                        )
                        if dt_i == DT - 1:
                            nc.vector.tensor_copy(
                                kall[h][:, cr * NF:(cr + 1) * NF], st["ps"][:])
                    k_q.append(u)

            for h, cr in [(2, 2), (2, 3), (3, 0), (3, 1), (3, 2), (3, 3)]:
                make_k_units(h, cr)

            # ---- Wo chain queue ----
            wo_q = deque()

            # GpSimd cannot read PSUM; during the groups all bounce casts
            # stay on DVE (an ACT Copy would delay the exp wall), in the
            # tail they alternate DVE / ScalarE (no exps left there)
            wo_cast_i = [0]

            def emit_wo(qhb, tt_i, dc, pool, alternate=False):
                dma_eng = nc.gpsimd if (alternate and wo_cast_i[0] % 2 == 0) \
                    else nc.sync
                tl = (tt_i - qhb * (TT // 2)) * P
                pw = pool.tile([P, NF], F32, name="pw", tag="w")
                for h in range(HPC):
                    nc.tensor.matmul(
                        pw[:], ot[(h, qhb)][:, tl:tl + P],
                        wo_sb[h][:, dc * NF:(dc + 1) * NF],
                        start=(h == 0), stop=(h == HPC - 1),
                    )
                ob = obounce.tile([P, NF], BF16, name="ob", tag="ob", bufs=6)
                # tail: alternate DVE/ScalarE; groups: mostly DVE with every
                # 3rd on ScalarE (it has ~3us/group slack under the exp wall)
                period = 2 if alternate else 3
                if wo_cast_i[0] % period == period - 1:
                    nc.scalar.activation(ob[:], pw[:], ACTF.Copy)
                else:
                    nc.vector.tensor_copy(ob[:], pw[:])
                wo_cast_i[0] += 1
                dma_eng.dma_start(
                    out_d[tt_i * P:(tt_i + 1) * P,
                          dc * NF:(dc + 1) * NF], ob[:])

            k_done = [0]

            def pop_filler():
                if k_q:
                    k_q.popleft()()
                    k_done[0] += 1
                elif wo_q:
                    qhb, tt_i, dc = wo_q.popleft()
                    emit_wo(qhb, tt_i, dc, ps_w)

            # emission-order deadlines: S(gi, kt) must be emitted AFTER the
            # deferred K units that produce its kall columns.  Unit counts:
            # h2cr2=16, h2cr3=32, h3cr0=48, h3cr1=64, h3cr2=80, h3cr3=96.
            def k_need(gi, kt_i):
                if gi == 2 and kt_i >= 8:        # (qh0, h2)
                    return 16 * (1 + (kt_i - 8) // 4)
                if gi == 3:                      # (qh0, h3)
                    return 48 + 16 * (kt_i // 4)
                return 0

            def drain_k_to(req):
                while k_done[0] < req and k_q:
                    k_q.popleft()()
                    k_done[0] += 1

            groups = [(qh, h) for qh in range(NQH) for h in range(HPC)]
            es: dict = {}
            esums: dict = {}

            def s_step(gi, kt_i, ps_s):
                qh, h = groups[gi]
                qs = qh * QH
                ps = ps_s.tile([P, QH], F32, name="ps_st", tag="s")
                for hf in range(2):
                    nc.tensor.matmul(
                        ps[:, hf * NF:(hf + 1) * NF],
                        kall[h][:, kt_i * P:(kt_i + 1) * P],
                        qall[h][:, qs + hf * NF:qs + (hf + 1) * NF],
                        start=True, stop=True,
                    )
                e = e_pool.tile([P, QH], BF16, name="e", tag="e", bufs=5)
                nc.scalar.activation(e[:], ps[:], ACTF.Exp)
                es[(gi, kt_i)] = e
                # accumulate the softmax denominator as soon as the exp tile
                # exists (DVE only: GpSimd tensor ops are ~3x slower)
                if kt_i == 0:
                    esum = es_pool.tile([P, QH], BF16, name="esum", tag="es", bufs=4)
                    esums[gi] = esum
                    nc.vector.tensor_copy(esum[:], e[:])
                else:
                    nc.vector.tensor_add(esums[gi][:], esums[gi][:], e[:])

            with ExitStack() as gctx:
                ps_s = gctx.enter_context(tc.tile_pool(name="ps_s", bufs=2, space="PSUM"))
                ps_o = gctx.enter_context(tc.tile_pool(name="ps_o", bufs=1, space="PSUM"))

                for gi, (qh, h) in enumerate(groups):
                    po = ps_o.tile([P, QH], F32, name="po")
                    # software-pipelined kt loop: S/exp leads PV by LEAD
                    for step in range(KT + LEAD):
                        if step < KT:
                            drain_k_to(k_need(gi, step))
                            s_step(gi, step, ps_s)
                        kt_j = step - LEAD
                        if kt_j >= 0:
                            e = es.pop((gi, kt_j))
                            for hf in range(2):
                                nc.tensor.matmul(
                                    po[:, hf * NF:(hf + 1) * NF],
                                    vall[kt_j][:, h * HD:(h + 1) * HD],
                                    e[:, hf * NF:(hf + 1) * NF],
                                    start=(kt_j == 0), stop=(kt_j == KT - 1),
                                )
                        # filler pacing: groups 0-3 absorb the deferred K
                        # units PE-bound (~21 units/group); groups 4-7
                        # interleave Wo chains every other step
                        if gi < 4:
                            pop_filler()
                            if step % 6 == 5:
                                pop_filler()
                        elif step % 2 == 1:
                            pop_filler()
                    # denominator AFTER the PV tail so the PE queue doesn't
                    # head-of-line block on the DVE esum chain (K units are
                    # small, so pop several to cover the ~1us latency)
                    for _ in range(4 if k_q else 2):
                        pop_filler()
                    pd = ps_s.tile([P, QH], F32, name="pd", tag="s")
                    for hf in range(2):
                        nc.tensor.matmul(
                            pd[:, hf * NF:(hf + 1) * NF], ones128[:],
                            esums[gi][:, hf * NF:(hf + 1) * NF],
                            start=True, stop=True,
                        )
                    rb = r_pool.tile([P, QH], F32, name="rb", tag="rb", bufs=2)
                    nc.vector.reciprocal_approx_fast(rb[:], pd[:])
                    pop_filler()
                    nc.vector.tensor_mul(ot[(h, qh)][:], po[:], rb[:])
                    # unlock Wo chains once a q-block's last head is done
                    if gi == HPC - 1:
                        for tt_i in range(TT // 2):
                            for dc in range(T // NF):
                                wo_q.append((0, tt_i, dc))
                    elif gi == 2 * HPC - 1:
                        for tt_i in range(TT // 2, TT):
                            for dc in range(T // NF):
                                wo_q.append((1, tt_i, dc))

                # overlap the pool-transition drain: first tail chains still
                # run from ps_w while ps_s/ps_o release
                for _ in range(6):
                    if wo_q:
                        qhb, tt_i, dc = wo_q.popleft()
                        emit_wo(qhb, tt_i, dc, ps_w, alternate=True)

            # ---- tail: remaining Wo chains straight from ps_w (no second
            # pool: the alternating DVE/ScalarE casts keep the 2-buf rotation
            # stall-free and skip the pool-transition drain); the last two
            # chains cast+DMA in halves so the final bytes leave earlier ----
            if True:
                ps_w2 = ps_w
                while len(wo_q) > 2:
                    qhb, tt_i, dc = wo_q.popleft()
                    emit_wo(qhb, tt_i, dc, ps_w2, alternate=True)
                while wo_q:
                    qhb, tt_i, dc = wo_q.popleft()
                    tl = (tt_i - qhb * (TT // 2)) * P
                    pw = ps_w2.tile([P, NF], F32, name="pw", tag="w")
                    for h in range(HPC):
                        nc.tensor.matmul(
                            pw[:], ot[(h, qhb)][:, tl:tl + P],
                            wo_sb[h][:, dc * NF:(dc + 1) * NF],
                            start=(h == 0), stop=(h == HPC - 1),
                        )
                    for hf in range(2):
                        obh = obounce.tile([P, NF // 2], BF16, name="obh",
                                           tag="obh", bufs=4)
                        if hf == 0:
                            nc.vector.tensor_copy(
                                obh[:], pw[:, 0:NF // 2])
                        else:
                            nc.scalar.activation(
                                obh[:], pw[:, NF // 2:NF], ACTF.Copy)
                        heng = nc.gpsimd if hf == 0 else nc.sync
                        heng.dma_start(
                            out_d[tt_i * P:(tt_i + 1) * P,
                                  dc * NF + hf * (NF // 2):
                                  dc * NF + (hf + 1) * (NF // 2)], obh[:])

    nc.compile()
    _CACHE["nc"] = nc
    return nc


def make_in_maps(hidden_states, Wq, Wk, Wv, Wo, **kwargs):
    bf = ml_dtypes.bfloat16
    hidden_states = np.asarray(hidden_states, np.float32)
    # fold the attention scale into Wq so no scaling is needed on-device
    Wq = np.asarray(Wq, np.float32) * SCALE
    Wk, Wv, Wo = (np.asarray(a, np.float32) for a in (Wk, Wv, Wo))
    in_maps = []
    for c in range(NCORES):
        b, g = divmod(c, CPB)
        cols = slice(g * HS, (g + 1) * HS)
        in_maps.append(dict(
            xT=np.ascontiguousarray(hidden_states[b].T).astype(bf),
            wq=np.ascontiguousarray(Wq[:, cols]).astype(bf),
            wk=np.ascontiguousarray(Wk[:, cols]).astype(bf),
            wv=np.ascontiguousarray(Wv[:, cols]).astype(bf),
            wo=np.ascontiguousarray(Wo[cols, :]).astype(bf),
        ))
    return in_maps


def gather(results):
    out = np.zeros((B, T, D), np.float32)
    for c in range(NCORES):
        out[c // CPB] += np.asarray(results[c]["out"], np.float32)
    return out


def kernel(hidden_states, Wq, Wk, Wv, Wo, Wd=None, bd=None, **kwargs):
    nc = _build()
    in_maps = make_in_maps(hidden_states, Wq, Wk, Wv, Wo)
    res = run_bass_kernel_spmd(nc, in_maps, core_ids=list(range(NCORES)))
    return gather(res.results)


# revision 40
# speedup vs baseline: 1.0134x; 1.0134x over previous
"""nn_DMSAttentionWrapper kernel for Trainium2, 8 NeuronCores.

The reference's eviction/causal mask is `jnp.maximum(dms, causal)` where the
two -inf regions are disjoint (dms: q > k+WIN and evicted; causal: q < k), so
the combined additive mask is identically zero: the oracle computes *dense,
non-causal, unmasked* multi-head attention.  The decision head (Wd, bd) does
not affect the output at all.

Sharding: data-parallel over batch (2) x tensor-parallel over heads (4 groups
of 4 heads).  Per-core schedule (latency-shaped around two facts: the ScalarE
exp wall of ~18us per (q-chunk, head) group vs ~14.6us of S+PV matmul work,
and a per-core input stream of only ~210GB/s because core pairs share a
device's HBM):

  Phase A: 8 warmup matmuls ramp the PE p-state while the first DMAs land
  (xt via the GpSimd queue in parallel with weights on Sync; xt0 split in 4
  chunks).  Q/K projection chains 0-9 are TWO-PASS (dt 0-7 into a bf16
  partial, dt 8-15 merged on DVE) so resident re-reads fill the xt arrival
  window instead of stalling the 4-wide dt-major ramp; K chains for
  (h2, qh1) and all of h3 are deferred into phase B.  V projection reuses
  the same psum pool/tag (no pool-transition drain).

  Phase B: per group: S^T into 2-bank PSUM, one exp ACTIVATE over
  [128,1024], exp tiles accumulated into the softmax denominator on DVE
  (GpSimd tensor ops are ~3x slower and cannot read PSUM), ones-matmul
  denominator AFTER the PV tail (no head-of-line block on the esum chain),
  reciprocal_approx_fast, normalize on DVE.  The exp-wall PE idle of groups
  0-3 is filled with the deferred K units (with emission-order deadline
  drains); groups 4-7 interleave block-0 Wo chains every other step (casts
  mostly DVE, every 3rd on ScalarE which has slack under the exp wall); the
  block-1 Wo tail runs with 4 psum bufs after releasing the group pools
  (bridged by 6 chains to hide the transition), casts alternating
  DVE/ScalarE, and the last two chains cast+DMA in halves to cut the end
  drain.  Host sums 4 bf16 partials per batch.
SCALE is folded into Wq on the host.
"""

import numpy as np
from collections import deque
from contextlib import ExitStack

import ml_dtypes
import concourse.bass as bass
import concourse.tile as tile
from concourse import bacc, mybir
from concourse.bass_utils import run_bass_kernel_spmd

B, T, D, H = 2, 2048, 2048, 16
HD = 128
NCORES = 8
CPB = NCORES // B          # cores per batch
HPC = H // CPB             # heads per core
HS = HPC * HD              # head-slice width (columns of Wq/Wk/Wv, rows of Wo)
SCALE = 1.0 / float(np.sqrt(HD))

F32 = mybir.dt.float32
BF16 = mybir.dt.bfloat16

P = 128                    # partition dim
NF = 512                   # matmul free dim / psum bank (fp32)
QH = 1024                  # q macro-chunk (exp ACTIVATE free dim, 2 psum banks)
NQH = T // QH              # 2 q macro-chunks
DT = D // P                # 16 contraction tiles over D
KT = T // P                # 16 k tiles
TT = T // P                # 16 t tiles
LEAD = 2                   # S/exp lead over PV/denom in the kt pipeline
NWARM = 8                  # warmup matmuls to ramp PE p-state during DMA wait

ALU = mybir.AluOpType
ACTF = mybir.ActivationFunctionType

_CACHE: dict = {}


def _build():
    if "nc" in _CACHE:
        return _CACHE["nc"]

    nc = bacc.Bacc("TRN2", target_bir_lowering=False, debug=False)

    xT = nc.dram_tensor("xT", [D, T], BF16, kind="ExternalInput").ap()
    wq_d = nc.dram_tensor("wq", [D, HS], BF16, kind="ExternalInput").ap()
    wk_d = nc.dram_tensor("wk", [D, HS], BF16, kind="ExternalInput").ap()
    wv_d = nc.dram_tensor("wv", [D, HS], BF16, kind="ExternalInput").ap()
    wo_d = nc.dram_tensor("wo", [HS, D], BF16, kind="ExternalInput").ap()
    out_d = nc.dram_tensor("out", [T, D], BF16, kind="ExternalOutput").ap()

    with tile.TileContext(nc) as tc, ExitStack() as ctx:
        const_pool = ctx.enter_context(tc.tile_pool(name="const", bufs=1))
        qk_pool = ctx.enter_context(tc.tile_pool(name="qk", bufs=1))
        v_pool = ctx.enter_context(tc.tile_pool(name="v", bufs=1))
        wo_pool = ctx.enter_context(tc.tile_pool(name="wo", bufs=1))
        xt_pool = ctx.enter_context(tc.tile_pool(name="xt", bufs=1))
        wk23_pool = ctx.enter_context(tc.tile_pool(name="wk23", bufs=1))

        # warm memset FIRST: the warmup matmuls depend only on it
        warm = const_pool.tile([P, NF], BF16, name="warm")
        nc.vector.memset(warm[:], 0.03125)
        ones_f32 = const_pool.tile([P, P], F32, name="ones_f32")
        nc.vector.memset(ones_f32[:], 1.0)
        ones128 = const_pool.tile([P, P], BF16, name="ones128")
        nc.vector.tensor_copy(ones128[:], ones_f32[:])

        # resident Q/K (head-dim x T, bf16) and V (T x 4*HD, bf16)
        qall = [qk_pool.tile([P, T], BF16, name=f"qh{h}") for h in range(HPC)]
        kall = [qk_pool.tile([P, T], BF16, name=f"kh{h}") for h in range(HPC)]
        vall = [v_pool.tile([P, HS], BF16, name=f"vt{i}") for i in range(KT)]
        # x^T tiles stay resident through phase B (deferred K chains read them)
        xt = [xt_pool.tile([P, T], BF16, name=f"xt{i}") for i in range(DT)]
        # wk columns for heads 2..3 also persist into phase B
        wk23 = [wk23_pool.tile([P, 2 * HD], BF16, name=f"wk23_{i}")
                for i in range(DT)]

        # =================== Phase A: projections ===================
        with ExitStack() as actx:
            w_pool = actx.enter_context(tc.tile_pool(name="w", bufs=1))
            ps_a = actx.enter_context(tc.tile_pool(name="ps_a", bufs=4, space="PSUM"))

            # warmup: ramp the PE p-state while the first input DMAs land
            # (depends only on the warm memset, not on ones128)
            for _ in range(NWARM):
                pswm = ps_a.tile([P, QH], F32, name="psqk", tag="a")
                nc.tensor.matmul(pswm[:, 0:NF], warm[:, 0:P], warm[:],
                                 start=True, stop=True)

            # xt DMAs on the GpSimd queue, weights on Sync: the two streams
            # transfer in parallel.  xt0 split in 4 chunks so the first ramp
            # chain's first matmul only waits for wq0 + 256KB of xt.
            for c in range(4):
                nc.gpsimd.dma_start(xt[0][:, c * NF:(c + 1) * NF],
                                    xT[0:P, c * NF:(c + 1) * NF])
            for i in range(1, DT):
                nc.gpsimd.dma_start(xt[i][:], xT[i * P:(i + 1) * P, :])

            wq_t, wk01_t, wv_t = [], [], []
            for i in range(DT):
                t = w_pool.tile([P, HS], BF16, name=f"wq{i}")
                nc.sync.dma_start(t[:], wq_d[i * P:(i + 1) * P, :])
                wq_t.append(t)
            for i in range(DT):
                t = w_pool.tile([P, 2 * HD], BF16, name=f"wk01_{i}")
                nc.sync.dma_start(t[:], wk_d[i * P:(i + 1) * P, 0:2 * HD])
                wk01_t.append(t)
            for i in range(DT):
                nc.sync.dma_start(wk23[i][:], wk_d[i * P:(i + 1) * P, 2 * HD:])
            for i in range(DT):
                t = w_pool.tile([P, HS], BF16, name=f"wv{i}")
                nc.sync.dma_start(t[:], wv_d[i * P:(i + 1) * P, :])
                wv_t.append(t)
            wo_sb = []
            for h in range(HPC):
                t = wo_pool.tile([P, D], BF16, name=f"wo{h}")
                nc.sync.dma_start(t[:], wo_d[h * HD:(h + 1) * HD, :])
                wo_sb.append(t)

            def wk_ap(dt_i, h):
                if h < 2:
                    return wk01_t[dt_i][:, h * HD:(h + 1) * HD]
                return wk23[dt_i][:, (h - 2) * HD:(h - 1) * HD]

            # Q/K projections: per (h, q-macro-chunk), one [128,1024] psum.
            # Only the K chain for (h3, qh1) is deferred into phase B.
            # The per-core input stream lands at only ~210GB/s (two cores
            # share a device's HBM), so xt tiles arrive ~2.4us apart while
            # 8 psum banks cap first-pass consumption at 1.71us/dt.  To
            # avoid pacing stalls, chains 0..SPLIT-1 are TWO-PASS: dt 0-7
            # into a bf16 partial, then dt 8-15 merged on DVE -- the
            # resident re-reads of part 1 fill the dt 8-15 arrival window.
            chains = [("q", h, qh) for h in range(HPC) for qh in range(NQH)]
            chains += [("k", 0, 0), ("k", 0, 1), ("k", 1, 0), ("k", 1, 1),
                       ("k", 2, 0)]
            SPLIT = 10
            HDT = DT // 2

            part_pool = actx.enter_context(tc.tile_pool(name="part", bufs=1))
            part1 = {}

            def qk_mm(ps, kind, h, qh, dt_i, start, stop):
                wt = wq_t[dt_i][:, h * HD:(h + 1) * HD] if kind == "q" \
                    else wk_ap(dt_i, h)
                for hf in range(2):
                    nc.tensor.matmul(
                        ps[:, hf * NF:(hf + 1) * NF],
                        wt,
                        xt[dt_i][:, qh * QH + hf * NF:qh * QH + (hf + 1) * NF],
                        start=start, stop=stop,
                    )

            def qk_dst(ci):
                kind, h, qh = chains[ci]
                dst = qall[h] if kind == "q" else kall[h]
                return dst[:, qh * QH:(qh + 1) * QH]

            # part-1 ramp: chains 0-3 dt-major over dt 0-7
            NRAMP = 4
            ramp_ps = [ps_a.tile([P, QH], F32, name="psqk", tag="a")
                       for _ in range(NRAMP)]
            for dt_i in range(HDT):
                for ci in range(NRAMP):
                    kind, h, qh = chains[ci]
                    qk_mm(ramp_ps[ci], kind, h, qh, dt_i,
                          start=(dt_i == 0), stop=(dt_i == HDT - 1))
            for ci in range(NRAMP):
                pt = part_pool.tile([P, QH], BF16, name=f"p1_{ci}")
                nc.vector.tensor_copy(pt[:], ramp_ps[ci][:])
                part1[ci] = pt

            # part-1 of chains 4..SPLIT-1 (dt 0-7 resident)
            for ci in range(NRAMP, SPLIT):
                kind, h, qh = chains[ci]
                ps = ps_a.tile([P, QH], F32, name="psqk", tag="a")
                for dt_i in range(HDT):
                    qk_mm(ps, kind, h, qh, dt_i,
                          start=(dt_i == 0), stop=(dt_i == HDT - 1))
                pt = part_pool.tile([P, QH], BF16, name=f"p1_{ci}")
                nc.vector.tensor_copy(pt[:], ps[:])
                part1[ci] = pt

            # full chains SPLIT..end (all xt resident by now)
            for ci in range(SPLIT, len(chains)):
                kind, h, qh = chains[ci]
                ps = ps_a.tile([P, QH], F32, name="psqk", tag="a")
                for dt_i in range(DT):
                    qk_mm(ps, kind, h, qh, dt_i,
                          start=(dt_i == 0), stop=(dt_i == DT - 1))
                nc.vector.tensor_copy(qk_dst(ci), ps[:])

            # part-2 of chains 0..SPLIT-1 (dt 8-15) + merge with part-1
            # (cast to bf16 first: a mixed f32-psum + bf16 tensor_tensor
            # takes a ~8.5us slow path on DVE)
            for ci in range(SPLIT):
                kind, h, qh = chains[ci]
                ps = ps_a.tile([P, QH], F32, name="psqk", tag="a")
                for dt_i in range(HDT, DT):
                    qk_mm(ps, kind, h, qh, dt_i,
                          start=(dt_i == HDT), stop=(dt_i == DT - 1))
                p2 = part_pool.tile([P, QH], BF16, name="p2", tag="p2", bufs=2)
                nc.vector.tensor_copy(p2[:], ps[:])
                nc.vector.tensor_add(qk_dst(ci), p2[:], part1[ci][:])

            # V projection reuses the same psum pool/tag (no pool-transition
            # drain; 4-deep rotation gives ample copy overlap)
            for tt_i in range(TT):
                ps = ps_a.tile([P, NF], F32, name="psv", tag="a")
                for dt_i in range(DT):
                    nc.tensor.matmul(
                        ps[:], xt[dt_i][:, tt_i * P:(tt_i + 1) * P],
                        wv_t[dt_i][:],
                        start=(dt_i == 0), stop=(dt_i == DT - 1),
                    )
                nc.vector.tensor_copy(vall[tt_i][:], ps[:])

        # =================== Phase B: attention + Wo ===================
        with ExitStack() as bctx:
            e_pool = bctx.enter_context(tc.tile_pool(name="e", bufs=5))
            es_pool = bctx.enter_context(tc.tile_pool(name="es", bufs=4))
            r_pool = bctx.enter_context(tc.tile_pool(name="r", bufs=2))
            ot_pool = bctx.enter_context(tc.tile_pool(name="ot", bufs=1))
            obounce = bctx.enter_context(tc.tile_pool(name="obounce", bufs=4))
            ps_w = bctx.enter_context(tc.tile_pool(name="ps_w", bufs=2, space="PSUM"))

            # one ot tile per (head, q-block) so Wo of block 0 never picks up
            # a false dependency on block-1 normalize writes
            ot = {(h, qh): ot_pool.tile([P, QH], BF16, name=f"ot{h}_{qh}")
                  for h in range(HPC) for qh in range(NQH)}

            # ---- deferred K projection units (fill exp-bound groups 0-3) ----
            # each (h, cr) is a 16-matmul chain into a [128,512] psum plus a
            # final copy into kall[h][:, cr*512 : (cr+1)*512]
            k_q = deque()

            def make_k_units(h, cr):
                st = {}
                for dt_i in range(DT):
                    def u(dt_i=dt_i, h=h, cr=cr, st=st):
                        if dt_i == 0:
                            st["ps"] = ps_w.tile([P, NF], F32, name="pw", tag="w")
                        nc.tensor.matmul(
                            st["ps"][:], wk23[dt_i][:, (h - 2) * HD:(h - 1) * HD],
                            xt[dt_i][:, cr * NF:(cr + 1) * NF],
                            start=(dt_i == 0), stop=(dt_i == DT - 1),
                        )
                        if dt_i == DT - 1:
                            nc.vector.tensor_copy(
                                kall[h][:, cr * NF:(cr + 1) * NF], st["ps"][:])
                    k_q.append(u)

            for h, cr in [(2, 2), (2, 3), (3, 0), (3, 1), (3, 2), (3, 3)]:
                make_k_units(h, cr)

            # ---- Wo chain queue ----
            wo_q = deque()

            # GpSimd cannot read PSUM; during the groups all bounce casts
            # stay on DVE (an ACT Copy would delay the exp wall), in the
            # tail they alternate DVE / ScalarE (no exps left there)
            wo_cast_i = [0]

            def emit_wo(qhb, tt_i, dc, pool, alternate=False):
                dma_eng = nc.gpsimd if (alternate and wo_cast_i[0] % 2 == 0) \
                    else nc.sync
                tl = (tt_i - qhb * (TT // 2)) * P
                pw = pool.tile([P, NF], F32, name="pw", tag="w")
                for h in range(HPC):
                    nc.tensor.matmul(
                        pw[:], ot[(h, qhb)][:, tl:tl + P],
                        wo_sb[h][:, dc * NF:(dc + 1) * NF],
                        start=(h == 0), stop=(h == HPC - 1),
                    )
                ob = obounce.tile([P, NF], BF16, name="ob", tag="ob", bufs=6)
                # tail: alternate DVE/ScalarE; groups: mostly DVE with every
                # 3rd on ScalarE (it has ~3us/group slack under the exp wall)
                period = 2 if alternate else 3
                if wo_cast_i[0] % period == period - 1:
                    nc.scalar.activation(ob[:], pw[:], ACTF.Copy)
                else:
                    nc.vector.tensor_copy(ob[:], pw[:])
                wo_cast_i[0] += 1
                dma_eng.dma_start(
                    out_d[tt_i * P:(tt_i + 1) * P,
                          dc * NF:(dc + 1) * NF], ob[:])

            k_done = [0]

            def pop_filler():
                if k_q:
                    k_q.popleft()()
                    k_done[0] += 1
                elif wo_q:
                    qhb, tt_i, dc = wo_q.popleft()
                    emit_wo(qhb, tt_i, dc, ps_w)

            # emission-order deadlines: S(gi, kt) must be emitted AFTER the
            # deferred K units that produce its kall columns.  Unit counts:
            # h2cr2=16, h2cr3=32, h3cr0=48, h3cr1=64, h3cr2=80, h3cr3=96.
            def k_need(gi, kt_i):
                if gi == 2 and kt_i >= 8:        # (qh0, h2)
                    return 16 * (1 + (kt_i - 8) // 4)
                if gi == 3:                      # (qh0, h3)
                    return 48 + 16 * (kt_i // 4)
                return 0

            def drain_k_to(req):
                while k_done[0] < req and k_q:
                    k_q.popleft()()
                    k_done[0] += 1

            groups = [(qh, h) for qh in range(NQH) for h in range(HPC)]
            es: dict = {}
            esums: dict = {}

            def s_step(gi, kt_i, ps_s):
                qh, h = groups[gi]
                qs = qh * QH
                ps = ps_s.tile([P, QH], F32, name="ps_st", tag="s")
                for hf in range(2):
                    nc.tensor.matmul(
                        ps[:, hf * NF:(hf + 1) * NF],
                        kall[h][:, kt_i * P:(kt_i + 1) * P],
                        qall[h][:, qs + hf * NF:qs + (hf + 1) * NF],
                        start=True, stop=True,
                    )
                e = e_pool.tile([P, QH], BF16, name="e", tag="e", bufs=5)
                nc.scalar.activation(e[:], ps[:], ACTF.Exp)
                es[(gi, kt_i)] = e
                # accumulate the softmax denominator as soon as the exp tile
                # exists (DVE only: GpSimd tensor ops are ~3x slower)
                if kt_i == 0:
                    esum = es_pool.tile([P, QH], BF16, name="esum", tag="es", bufs=4)
                    esums[gi] = esum
                    nc.vector.tensor_copy(esum[:], e[:])
                else:
                    nc.vector.tensor_add(esums[gi][:], esums[gi][:], e[:])

            with ExitStack() as gctx:
                ps_s = gctx.enter_context(tc.tile_pool(name="ps_s", bufs=2, space="PSUM"))
                ps_o = gctx.enter_context(tc.tile_pool(name="ps_o", bufs=1, space="PSUM"))

                for gi, (qh, h) in enumerate(groups):
                    po = ps_o.tile([P, QH], F32, name="po")
                    # software-pipelined kt loop: S/exp leads PV by LEAD
                    for step in range(KT + LEAD):
                        if step < KT:
                            drain_k_to(k_need(gi, step))
                            s_step(gi, step, ps_s)
                        kt_j = step - LEAD
                        if kt_j >= 0:
                            e = es.pop((gi, kt_j))
                            for hf in range(2):
                                nc.tensor.matmul(
                                    po[:, hf * NF:(hf + 1) * NF],
                                    vall[kt_j][:, h * HD:(h + 1) * HD],
                                    e[:, hf * NF:(hf + 1) * NF],
                                    start=(kt_j == 0), stop=(kt_j == KT - 1),
                                )
                        # filler pacing: groups 0-3 absorb the deferred K
                        # units PE-bound (~21 units/group); groups 4-7
                        # interleave Wo chains every other step
                        if gi < 4:
                            pop_filler()
                            if step % 6 == 5:
                                pop_filler()
                        elif step % 2 == 1:
                            pop_filler()
                    # denominator AFTER the PV tail so the PE queue doesn't
                    # head-of-line block on the DVE esum chain (K units are
                    # small, so pop several to cover the ~1us latency)
                    for _ in range(4 if k_q else 2):
                        pop_filler()
                    pd = ps_s.tile([P, QH], F32, name="pd", tag="s")
                    for hf in range(2):
                        nc.tensor.matmul(
                            pd[:, hf * NF:(hf + 1) * NF], ones128[:],
                            esums[gi][:, hf * NF:(hf + 1) * NF],
                            start=True, stop=True,
                        )
                    rb = r_pool.tile([P, QH], F32, name="rb", tag="rb", bufs=2)
                    nc.vector.reciprocal_approx_fast(rb[:], pd[:])
                    pop_filler()
                    nc.vector.tensor_mul(ot[(h, qh)][:], po[:], rb[:])
                    # unlock Wo chains once a q-block's last head is done
                    if gi == HPC - 1:
                        for tt_i in range(TT // 2):
                            for dc in range(T // NF):
                                wo_q.append((0, tt_i, dc))
                    elif gi == 2 * HPC - 1:
                        for tt_i in range(TT // 2, TT):
                            for dc in range(T // NF):
                                wo_q.append((1, tt_i, dc))

                # overlap the pool-transition drain: first tail chains still
                # run from ps_w while ps_s/ps_o release
                for _ in range(6):
                    if wo_q:
                        qhb, tt_i, dc = wo_q.popleft()
                        emit_wo(qhb, tt_i, dc, ps_w, alternate=True)

            # ---- tail: remaining Wo chains with 4 psum bufs; the last two
            # chains cast+DMA in halves so the final bytes leave earlier ----
            with tc.tile_pool(name="ps_w2", bufs=6, space="PSUM") as ps_w2:
                while len(wo_q) > 2:
                    qhb, tt_i, dc = wo_q.popleft()
                    emit_wo(qhb, tt_i, dc, ps_w2, alternate=True)
                while wo_q:
                    qhb, tt_i, dc = wo_q.popleft()
                    tl = (tt_i - qhb * (TT // 2)) * P
                    pw = ps_w2.tile([P, NF], F32, name="pw", tag="w")
                    for h in range(HPC):
                        nc.tensor.matmul(
                            pw[:], ot[(h, qhb)][:, tl:tl + P],
                            wo_sb[h][:, dc * NF:(dc + 1) * NF],
                            start=(h == 0), stop=(h == HPC - 1),
                        )
                    for hf in range(2):
                        obh = obounce.tile([P, NF // 2], BF16, name="obh",
                                           tag="obh", bufs=4)
                        if hf == 0:
                            nc.vector.tensor_copy(
                                obh[:], pw[:, 0:NF // 2])
                        else:
                            nc.scalar.activation(
                                obh[:], pw[:, NF // 2:NF], ACTF.Copy)
                        heng = nc.gpsimd if hf == 0 else nc.sync
                        heng.dma_start(
                            out_d[tt_i * P:(tt_i + 1) * P,
                                  dc * NF + hf * (NF // 2):
                                  dc * NF + (hf + 1) * (NF // 2)], obh[:])

    nc.compile()
    _CACHE["nc"] = nc
    return nc


def make_in_maps(hidden_states, Wq, Wk, Wv, Wo, **kwargs):
    bf = ml_dtypes.bfloat16
    hidden_states = np.asarray(hidden_states, np.float32)
    # fold the attention scale into Wq so no scaling is needed on-device
    Wq = np.asarray(Wq, np.float32) * SCALE
    Wk, Wv, Wo = (np.asarray(a, np.float32) for a in (Wk, Wv, Wo))
    in_maps = []
    for c in range(NCORES):
        b, g = divmod(c, CPB)
        cols = slice(g * HS, (g + 1) * HS)
        in_maps.append(dict(
            xT=np.ascontiguousarray(hidden_states[b].T).astype(bf),
            wq=np.ascontiguousarray(Wq[:, cols]).astype(bf),
            wk=np.ascontiguousarray(Wk[:, cols]).astype(bf),
            wv=np.ascontiguousarray(Wv[:, cols]).astype(bf),
            wo=np.ascontiguousarray(Wo[cols, :]).astype(bf),
        ))
    return in_maps


def gather(results):
    out = np.zeros((B, T, D), np.float32)
    for c in range(NCORES):
        out[c // CPB] += np.asarray(results[c]["out"], np.float32)
    return out


def kernel(hidden_states, Wq, Wk, Wv, Wo, Wd=None, bd=None, **kwargs):
    nc = _build()
    in_maps = make_in_maps(hidden_states, Wq, Wk, Wv, Wo)
    res = run_bass_kernel_spmd(nc, in_maps, core_ids=list(range(NCORES)))
    return gather(res.results)
